# revision 41
# baseline (speedup 1.0000x reference)
"""Trainium2 Bass kernel for bilinear-attention (weights softmax + context).

reference:
    qW = query @ W                      [B, Lq, D]
    scores = qW @ key^T + bias          [B, Lq, Lk]   (bias cancels in softmax)
    weights = softmax(scores, -1)       [B, Lq, Lk]
    ctx = weights @ value               [B, Lq, Dv]
    returns (weights, ctx)

Sharding: data-parallel over batch B=8 -> one batch element per NeuronCore.

Numerics per core:
  - qW^T: bf16 hi/lo 3-pass matmul (x ~ hi + lo; x@y ~ hh + hl + lh), fp32 PSUM.
  - scores: selectable S_MODE:
      "bf16x3": bf16 hi/lo 3-pass both sides (most accurate)
      "f32r2" : stationary qW^T split into two float32r parts, K^T single f32r
      "f32r1" : single-pass float32r (fastest)
  - softmax: chunk max (negated) -> exp(s - m) with accumulated Z -> 1/Z scale
  - ctx: single-pass bf16 (weights^T via PE transpose x V)
"""
import sys
import os

for _p in ("/opt/trn_rl_repo", "/root/.axon_site/_ro/trn_rl_repo"):
    if os.path.isdir(_p) and _p not in sys.path:
        sys.path.insert(0, _p)

import numpy as np
import concourse.bass as bass
import concourse.mybir as mybir
import concourse.tile as tile
from concourse import bacc
from concourse.bass_utils import run_bass_kernel_spmd
from concourse.masks import make_identity

f32 = mybir.dt.float32
f32r = mybir.dt.float32r
bf16 = mybir.dt.bfloat16
AF = mybir.ActivationFunctionType
AX = mybir.AxisListType
ALU = mybir.AluOpType

# Problem shape (hardcoded; one batch element per core)
B, LQ, LK, D, DV = 8, 2048, 2048, 1024, 1024
P = 128                      # partitions
DT = D // P                  # 8 d-tiles
ET = D // P                  # 8 e-tiles
KT = LK // P                 # 16 k-tiles
NQB = LQ // P                # 16 q-blocks
NSB = 4                      # q-superblocks (512 q each) for qW phase
SBQ = LQ // NSB              # 512
NKC = LK // 512              # 4 k-chunks of 512
NVC = DV // 512              # 2 v-chunks of 512

S_MODE = os.environ.get("S_MODE", "f32rall")


def _split_hilo(nc, hi_slice, lo_slice, src_f32, hi_on_scalar=False):
    """hi = round(src); lo = round(src - hi). src may be PSUM or SBUF f32."""
    if hi_on_scalar:
        nc.scalar.copy(hi_slice, src_f32)
    else:
        nc.vector.tensor_copy(hi_slice, src_f32)
    nc.vector.tensor_sub(lo_slice, src_f32, hi_slice)


def build_kernel(s_mode=None):
    s_mode = s_mode or S_MODE
    assert s_mode in ("bf16x3", "f32r2", "f32r1", "f32rall")
    qw_f32r = s_mode == "f32rall"       # qW single-pass f32r too

    nc = bacc.Bacc("TRN2", target_bir_lowering=False, debug=False)

    q_in = nc.declare_dram_parameter("query", [LQ, D], f32, isOutput=False)
    k_in = nc.declare_dram_parameter("key", [LK, D], f32, isOutput=False)
    v_in = nc.declare_dram_parameter("value", [LK, DV], f32, isOutput=False)
    w_in = nc.declare_dram_parameter("W", [D, D], f32, isOutput=False)
    nc.declare_dram_parameter("bias", [1], f32, isOutput=False)  # softmax-invariant
    wt_out = nc.declare_dram_parameter("weights", [LQ, LK], f32, isOutput=True)
    cx_out = nc.declare_dram_parameter("ctx", [LQ, DV], f32, isOutput=True)

    with tile.TileContext(nc) as tc:
        with tc.tile_pool(name="resident", bufs=1) as RP, \
             tc.tile_pool(name="qf", bufs=2) as QF, \
             tc.tile_pool(name="qt", bufs=1) as QT, \
             tc.tile_pool(name="ps_s", bufs=4, space="PSUM") as PS_S, \
             tc.tile_pool(name="ps_gen", bufs=4, space="PSUM") as PS_G:

            ident = RP.tile([P, P], f32)
            make_identity(nc, ident[:])
            identb = RP.tile([P, P], bf16)
            nc.vector.tensor_copy(identb[:], ident[:])

            # resident operand arrays
            if qw_f32r:
                wr = RP.tile([P, DT, D], f32r)   # W[d, e] (d on partitions)
            else:
                whi = RP.tile([P, DT, D], bf16)
                wlo = RP.tile([P, DT, D], bf16)
            if s_mode == "bf16x3":
                kthi = RP.tile([P, ET, LK], bf16)  # K^T[e, k] (e on partitions)
                ktlo = RP.tile([P, ET, LK], bf16)
            else:
                ktr = RP.tile([P, ET, LK], f32r)
            vb = RP.tile([P, KT, DV], bf16)      # V[k, v]   (k on partitions)

            # Q^T for one superblock (written per-superblock in main loop)
            if qw_f32r:
                qtr = QT.tile([P, DT, SBQ], f32r, tag="qtr")
            else:
                qthi = QT.tile([P, DT, SBQ], bf16, tag="qthi")
                qtlo = QT.tile([P, DT, SBQ], bf16, tag="qtlo")

            # ---------------- main ----------------
            with tc.tile_pool(name="qwt", bufs=1) as QWT, \
                 tc.tile_pool(name="wout", bufs=8) as WO, \
                 tc.tile_pool(name="ptp", bufs=5) as PTP, \
                 tc.tile_pool(name="cxp", bufs=2) as CXP, \
                 tc.tile_pool(name="stats", bufs=2) as ST, \
                 tc.tile_pool(name="setup", bufs=3) as SP:

                def emit_w_setup():
                    # W gates the qW matmuls of superblock 0 - do it first
                    for c in range(DT):
                        wf = SP.tile([P, 1, D], f32, tag="wvf")
                        nc.sync.dma_start(
                            wf[:],
                            w_in.rearrange("(t p) e -> p t e",
                                           p=P)[:, c:c + 1, :],
                        )
                        if qw_f32r:
                            nc.vector.tensor_copy(wr[:, c:c + 1, :], wf[:])
                        else:
                            _split_hilo(nc, whi[:, c:c + 1, :],
                                        wlo[:, c:c + 1, :], wf[:],
                                        hi_on_scalar=True)

                def emit_k_tiles(kts):
                    # K^T via PE transposes; copies on ACT
                    for kt in kts:
                        kf = QF.tile([P, D], f32, tag="qf")
                        nc.sync.dma_start(kf[:], k_in[kt * P:(kt + 1) * P, :])
                        for g in range(2):  # groups of 4 e-tiles
                            pt = PS_G.tile([P, 512], f32, tag="gen")
                            for j in range(4):
                                et = g * 4 + j
                                nc.tensor.transpose(
                                    pt[:, j * P:(j + 1) * P],
                                    kf[:, et * P:(et + 1) * P],
                                    ident[:],
                                )
                            src = pt[:].rearrange("p (a b) -> p a b", a=4)
                            sl = (slice(None), slice(g * 4, (g + 1) * 4),
                                  slice(kt * P, (kt + 1) * P))
                            if s_mode == "bf16x3":
                                _split_hilo(nc, kthi[sl], ktlo[sl], src,
                                            hi_on_scalar=True)
                            else:
                                nc.scalar.copy(ktr[sl], src)

                def emit_v_setup():
                    # V rounded to bf16 on DVE (needed only at first ctx)
                    for c in range(KT):
                        vf = SP.tile([P, 1, DV], f32, tag="wvf")
                        nc.sync.dma_start(
                            vf[:],
                            v_in.rearrange("(t p) v -> p t v",
                                           p=P)[:, c:c + 1, :],
                        )
                        nc.vector.tensor_copy(vb[:, c:c + 1, :], vf[:])

                if s_mode == "bf16x3":
                    qwhi = QWT.tile([P, ET, SBQ], bf16, tag="qwhi")
                    qwlo = QWT.tile([P, ET, SBQ], bf16, tag="qwlo")
                elif s_mode == "f32r2":
                    qwhi = QWT.tile([P, ET, SBQ], f32r, tag="qwhi")
                    qwlo = QWT.tile([P, ET, SBQ], f32r, tag="qwlo")
                else:
                    qwr = QWT.tile([P, ET, SBQ], f32r, tag="qwr")

                def emit_q_phase(sb):
                    # Q^T (hi/lo) for superblock sb
                    for qt in range(4):
                        qb = sb * 4 + qt
                        qf = QF.tile([P, D], f32, tag="qf")
                        nc.sync.dma_start(qf[:], q_in[qb * P:(qb + 1) * P, :])
                        for g in range(2):
                            pt = PS_G.tile([P, 512], f32, tag="gen")
                            for j in range(4):
                                dt = g * 4 + j
                                nc.tensor.transpose(
                                    pt[:, j * P:(j + 1) * P],
                                    qf[:, dt * P:(dt + 1) * P],
                                    ident[:],
                                )
                            src = pt[:].rearrange("p (a b) -> p a b", a=4)
                            sl = (slice(None), slice(g * 4, (g + 1) * 4),
                                  slice(qt * P, (qt + 1) * P))
                            if qw_f32r:
                                nc.vector.tensor_copy(qtr[sl], src)
                            else:
                                _split_hilo(nc, qthi[sl], qtlo[sl], src,
                                            hi_on_scalar=True)

                def emit_finish(st_):
                    # deferred tail of a q-block. P^T is taken from the
                    # UNNORMALIZED exp chunks (no invz dependency); the 1/Z
                    # scale folds into the ctx PSUM->SBUF copy. The weights
                    # chunks are scaled in place afterwards for their DMA.
                    qb, wchunks, invz = st_
                    ptiles = []
                    for kc in range(NKC):
                        wo = wchunks[kc]
                        pt = PS_G.tile([P, 512], f32, tag="gen")
                        for j in range(4):
                            nc.tensor.transpose(
                                pt[:, j * P:(j + 1) * P],
                                wo[:, j * P:(j + 1) * P],
                                ident[:],
                            )
                        ptt = PTP.tile([P, 512], bf16, tag="pt")
                        nc.scalar.copy(ptt[:], pt[:])
                        ptiles.append(ptt)

                    for vc in range(NVC):
                        pc = PS_G.tile([P, 512], f32, tag="gen")
                        for kt in range(KT):
                            lhsT = ptiles[kt // 4][:, (kt % 4) * P:
                                                   (kt % 4 + 1) * P]
                            nc.tensor.matmul(
                                pc[:], lhsT,
                                vb[:, kt, vc * 512:(vc + 1) * 512],
                                start=(kt == 0), stop=(kt == KT - 1),
                            )
                        cx = CXP.tile([P, 512], f32, tag="cx")
                        nc.vector.tensor_scalar_mul(cx[:], pc[:], invz[:])
                        nc.sync.dma_start(
                            cx_out[qb * P:(qb + 1) * P,
                                   vc * 512:(vc + 1) * 512],
                            cx[:],
                        )

                    for kc in range(NKC):
                        wo = wchunks[kc]
                        nc.vector.tensor_scalar_mul(wo[:], wo[:], invz[:])
                        nc.sync.dma_start(
                            wt_out[qb * P:(qb + 1) * P,
                                   kc * 512:(kc + 1) * 512],
                            wo[:],
                        )

                pending = None
                emit_q_phase(0)
                emit_k_tiles(range(0, 4))
                emit_w_setup()
                for sb in range(NSB):
                    # -- qW^T[e, q] for this superblock --
                    for et in range(ET):
                        pq = PS_G.tile([P, SBQ], f32, tag="gen")
                        for dt in range(DT):
                            if qw_f32r:
                                mms = [(wr[:, dt, et * P:(et + 1) * P],
                                        qtr[:, dt, :])]
                            else:
                                lw_hi = whi[:, dt, et * P:(et + 1) * P]
                                lw_lo = wlo[:, dt, et * P:(et + 1) * P]
                                mms = [(lw_hi, qthi[:, dt, :]),
                                       (lw_hi, qtlo[:, dt, :]),
                                       (lw_lo, qthi[:, dt, :])]
                            for mi, (l, r) in enumerate(mms):
                                nc.tensor.matmul(
                                    pq[:], l, r,
                                    start=(dt == 0 and mi == 0),
                                    stop=(dt == DT - 1 and mi == len(mms) - 1),
                                )
                        if s_mode in ("bf16x3", "f32r2"):
                            _split_hilo(nc, qwhi[:, et, :], qwlo[:, et, :],
                                        pq[:], hi_on_scalar=True)
                        else:
                            nc.vector.tensor_copy(qwr[:, et, :], pq[:])

                    if sb == 0:
                        # rest of K + V land while qW(0) runs on PE
                        emit_k_tiles(range(4, KT))
                        emit_v_setup()

                    # -- per q-block: S, softmax, weights, P^T, ctx --
                    for qt in range(4):
                        qb = sb * 4 + qt
                        qq = qt * P

                        schunks = []
                        negmax = ST.tile([P, NKC], f32, tag="negmax")
                        for kc in range(NKC):
                            ps = PS_S.tile([P, 512], f32, tag="schunk")
                            ksl = (slice(None), None,
                                   slice(kc * 512, (kc + 1) * 512))
                            for et in range(ET):
                                if s_mode == "bf16x3":
                                    r_hi = kthi[:, et, kc * 512:(kc + 1) * 512]
                                    r_lo = ktlo[:, et, kc * 512:(kc + 1) * 512]
                                    mms = [(qwhi[:, et, qq:qq + P], r_hi),
                                           (qwhi[:, et, qq:qq + P], r_lo),
                                           (qwlo[:, et, qq:qq + P], r_hi)]
                                elif s_mode == "f32r2":
                                    r = ktr[:, et, kc * 512:(kc + 1) * 512]
                                    mms = [(qwhi[:, et, qq:qq + P], r),
                                           (qwlo[:, et, qq:qq + P], r)]
                                else:
                                    r = ktr[:, et, kc * 512:(kc + 1) * 512]
                                    mms = [(qwr[:, et, qq:qq + P], r)]
                                for mi, (l, r_) in enumerate(mms):
                                    nc.tensor.matmul(
                                        ps[:], l, r_,
                                        start=(et == 0 and mi == 0),
                                        stop=(et == ET - 1 and
                                              mi == len(mms) - 1),
                                    )
                            nc.vector.reduce_max(negmax[:, kc:kc + 1], ps[:],
                                                 axis=AX.X, negate=True)
                            schunks.append(ps)

                        # row stats:  -m = min(negmax);  Z = sum exp(s - m)
                        nmin = ST.tile([P, 1], f32, tag="nmin")
                        nc.vector.tensor_reduce(nmin[:], negmax[:], axis=AX.X,
                                                op=ALU.min)
                        zparts = ST.tile([P, NKC], f32, tag="zparts")
                        wchunks = []
                        for kc in range(NKC):
                            wo = WO.tile([P, 512], f32, tag="wout")
                            nc.scalar.activation(wo[:], schunks[kc][:], AF.Exp,
                                                 bias=nmin[:], scale=1.0,
                                                 accum_out=zparts[:, kc:kc + 1])
                            wchunks.append(wo)
                        z = ST.tile([P, 1], f32, tag="z")
                        nc.vector.reduce_sum(z[:], zparts[:], axis=AX.X)
                        invz = ST.tile([P, 1], f32, tag="invz")
                        nc.vector.reciprocal(invz[:], z[:])

                        # finish the PREVIOUS block while this one's softmax
                        # runs on DVE/ACT (keeps PE fed)
                        if pending is not None:
                            emit_finish(pending)
                        pending = (qb, wchunks, invz)

                        if qt == 0 and sb + 1 < NSB:
                            emit_q_phase(sb + 1)

                emit_finish(pending)

    nc.compile()
    return nc


_NC_CACHE = None


def _get_nc():
    global _NC_CACHE
    if _NC_CACHE is None:
        _NC_CACHE = build_kernel()
    return _NC_CACHE


def kernel(**inputs):
    q = np.ascontiguousarray(np.asarray(inputs["query"], dtype=np.float32))
    k = np.ascontiguousarray(np.asarray(inputs["key"], dtype=np.float32))
    v = np.ascontiguousarray(np.asarray(inputs["value"], dtype=np.float32))
    W = np.ascontiguousarray(np.asarray(inputs["W"], dtype=np.float32))
    bias = np.ascontiguousarray(np.asarray(inputs["bias"], dtype=np.float32))

    nc = _get_nc()
    in_maps = [
        {"query": q[i], "key": k[i], "value": v[i], "W": W, "bias": bias}
        for i in range(B)
    ]
    res = run_bass_kernel_spmd(nc, in_maps, core_ids=list(range(B)))
    weights = np.stack([res.results[i]["weights"] for i in range(B)])
    ctx = np.stack([res.results[i]["ctx"] for i in range(B)])
    return (weights, ctx)


if __name__ == "__main__":
    nc = build_kernel()
    print("kernel built ok")


# revision 42
# speedup vs baseline: 1.0062x; 1.0062x over previous
"""Trainium2 Bass kernel for bilinear-attention (weights softmax + context).

reference:
    qW = query @ W                      [B, Lq, D]
    scores = qW @ key^T + bias          [B, Lq, Lk]   (bias cancels in softmax)
    weights = softmax(scores, -1)       [B, Lq, Lk]
    ctx = weights @ value               [B, Lq, Dv]
    returns (weights, ctx)

Sharding: data-parallel over batch B=8 -> one batch element per NeuronCore.

Numerics per core:
  - qW^T: bf16 hi/lo 3-pass matmul (x ~ hi + lo; x@y ~ hh + hl + lh), fp32 PSUM.
  - scores: selectable S_MODE:
      "bf16x3": bf16 hi/lo 3-pass both sides (most accurate)
      "f32r2" : stationary qW^T split into two float32r parts, K^T single f32r
      "f32r1" : single-pass float32r (fastest)
  - softmax: chunk max (negated) -> exp(s - m) with accumulated Z -> 1/Z scale
  - ctx: single-pass bf16 (weights^T via PE transpose x V)
"""
import sys
import os

for _p in ("/opt/trn_rl_repo", "/root/.axon_site/_ro/trn_rl_repo"):
    if os.path.isdir(_p) and _p not in sys.path:
        sys.path.insert(0, _p)

import numpy as np
import concourse.bass as bass
import concourse.mybir as mybir
import concourse.tile as tile
from concourse import bacc
from concourse.bass_utils import run_bass_kernel_spmd
from concourse.masks import make_identity

f32 = mybir.dt.float32
f32r = mybir.dt.float32r
bf16 = mybir.dt.bfloat16
AF = mybir.ActivationFunctionType
AX = mybir.AxisListType
ALU = mybir.AluOpType

# Problem shape (hardcoded; one batch element per core)
B, LQ, LK, D, DV = 8, 2048, 2048, 1024, 1024
P = 128                      # partitions
DT = D // P                  # 8 d-tiles
ET = D // P                  # 8 e-tiles
KT = LK // P                 # 16 k-tiles
NQB = LQ // P                # 16 q-blocks
NSB = 4                      # q-superblocks (512 q each) for qW phase
SBQ = LQ // NSB              # 512
NKC = LK // 512              # 4 k-chunks of 512
NVC = DV // 512              # 2 v-chunks of 512

S_MODE = os.environ.get("S_MODE", "f32rall")


def _split_hilo(nc, hi_slice, lo_slice, src_f32, hi_on_scalar=False):
    """hi = round(src); lo = round(src - hi). src may be PSUM or SBUF f32."""
    if hi_on_scalar:
        nc.scalar.copy(hi_slice, src_f32)
    else:
        nc.vector.tensor_copy(hi_slice, src_f32)
    nc.vector.tensor_sub(lo_slice, src_f32, hi_slice)


def build_kernel(s_mode=None):
    s_mode = s_mode or S_MODE
    assert s_mode in ("bf16x3", "f32r2", "f32r1", "f32rall")
    qw_f32r = s_mode == "f32rall"       # qW single-pass f32r too

    nc = bacc.Bacc("TRN2", target_bir_lowering=False, debug=False)

    q_in = nc.declare_dram_parameter("query", [LQ, D], f32, isOutput=False)
    k_in = nc.declare_dram_parameter("key", [LK, D], f32, isOutput=False)
    v_in = nc.declare_dram_parameter("value", [LK, DV], f32, isOutput=False)
    w_in = nc.declare_dram_parameter("W", [D, D], f32, isOutput=False)
    nc.declare_dram_parameter("bias", [1], f32, isOutput=False)  # softmax-invariant
    wt_out = nc.declare_dram_parameter("weights", [LQ, LK], f32, isOutput=True)
    cx_out = nc.declare_dram_parameter("ctx", [LQ, DV], f32, isOutput=True)

    with tile.TileContext(nc) as tc:
        with tc.tile_pool(name="resident", bufs=1) as RP, \
             tc.tile_pool(name="qf", bufs=2) as QF, \
             tc.tile_pool(name="qt", bufs=1) as QT, \
             tc.tile_pool(name="ps_s", bufs=5, space="PSUM") as PS_S, \
             tc.tile_pool(name="ps_gen", bufs=3, space="PSUM") as PS_G:

            ident = RP.tile([P, P], f32)
            make_identity(nc, ident[:])
            identb = RP.tile([P, P], bf16)
            nc.vector.tensor_copy(identb[:], ident[:])

            # resident operand arrays
            if qw_f32r:
                wr = RP.tile([P, DT, D], f32r)   # W[d, e] (d on partitions)
            else:
                whi = RP.tile([P, DT, D], bf16)
                wlo = RP.tile([P, DT, D], bf16)
            if s_mode == "bf16x3":
                kthi = RP.tile([P, ET, LK], bf16)  # K^T[e, k] (e on partitions)
                ktlo = RP.tile([P, ET, LK], bf16)
            else:
                ktr = RP.tile([P, ET, LK], f32r)
            vb = RP.tile([P, KT, DV], bf16)      # V[k, v]   (k on partitions)

            # Q^T for one superblock (written per-superblock in main loop)
            if qw_f32r:
                qtr = QT.tile([P, DT, SBQ], f32r, tag="qtr")
            else:
                qthi = QT.tile([P, DT, SBQ], bf16, tag="qthi")
                qtlo = QT.tile([P, DT, SBQ], bf16, tag="qtlo")

            # ---------------- main ----------------
            with tc.tile_pool(name="qwt", bufs=1) as QWT, \
                 tc.tile_pool(name="wout", bufs=8) as WO, \
                 tc.tile_pool(name="ptp", bufs=5) as PTP, \
                 tc.tile_pool(name="cxp", bufs=2) as CXP, \
                 tc.tile_pool(name="stats", bufs=2) as ST, \
                 tc.tile_pool(name="setup", bufs=3) as SP:

                def emit_w_setup():
                    # W gates the qW matmuls of superblock 0 - do it first
                    for c in range(DT):
                        wf = SP.tile([P, 1, D], f32, tag="wvf")
                        nc.sync.dma_start(
                            wf[:],
                            w_in.rearrange("(t p) e -> p t e",
                                           p=P)[:, c:c + 1, :],
                        )
                        if qw_f32r:
                            nc.vector.tensor_copy(wr[:, c:c + 1, :], wf[:])
                        else:
                            _split_hilo(nc, whi[:, c:c + 1, :],
                                        wlo[:, c:c + 1, :], wf[:],
                                        hi_on_scalar=True)

                def emit_k_tiles(kts):
                    # K^T via PE transposes; copies on ACT
                    for kt in kts:
                        kf = QF.tile([P, D], f32, tag="qf")
                        nc.sync.dma_start(kf[:], k_in[kt * P:(kt + 1) * P, :])
                        for g in range(2):  # groups of 4 e-tiles
                            pt = PS_G.tile([P, 512], f32, tag="gen")
                            for j in range(4):
                                et = g * 4 + j
                                nc.tensor.transpose(
                                    pt[:, j * P:(j + 1) * P],
                                    kf[:, et * P:(et + 1) * P],
                                    ident[:],
                                )
                            src = pt[:].rearrange("p (a b) -> p a b", a=4)
                            sl = (slice(None), slice(g * 4, (g + 1) * 4),
                                  slice(kt * P, (kt + 1) * P))
                            if s_mode == "bf16x3":
                                _split_hilo(nc, kthi[sl], ktlo[sl], src,
                                            hi_on_scalar=True)
                            else:
                                nc.scalar.copy(ktr[sl], src)

                def emit_v_setup():
                    # V rounded to bf16 on DVE (needed only at first ctx)
                    for c in range(KT):
                        vf = SP.tile([P, 1, DV], f32, tag="wvf")
                        nc.sync.dma_start(
                            vf[:],
                            v_in.rearrange("(t p) v -> p t v",
                                           p=P)[:, c:c + 1, :],
                        )
                        nc.vector.tensor_copy(vb[:, c:c + 1, :], vf[:])

                if s_mode == "bf16x3":
                    qwhi = QWT.tile([P, ET, SBQ], bf16, tag="qwhi")
                    qwlo = QWT.tile([P, ET, SBQ], bf16, tag="qwlo")
                elif s_mode == "f32r2":
                    qwhi = QWT.tile([P, ET, SBQ], f32r, tag="qwhi")
                    qwlo = QWT.tile([P, ET, SBQ], f32r, tag="qwlo")
                else:
                    qwr = QWT.tile([P, ET, SBQ], f32r, tag="qwr")

                def emit_q_phase(sb):
                    # Q^T (hi/lo) for superblock sb
                    for qt in range(4):
                        qb = sb * 4 + qt
                        qf = QF.tile([P, D], f32, tag="qf")
                        nc.sync.dma_start(qf[:], q_in[qb * P:(qb + 1) * P, :])
                        for g in range(2):
                            pt = PS_G.tile([P, 512], f32, tag="gen")
                            for j in range(4):
                                dt = g * 4 + j
                                nc.tensor.transpose(
                                    pt[:, j * P:(j + 1) * P],
                                    qf[:, dt * P:(dt + 1) * P],
                                    ident[:],
                                )
                            src = pt[:].rearrange("p (a b) -> p a b", a=4)
                            sl = (slice(None), slice(g * 4, (g + 1) * 4),
                                  slice(qt * P, (qt + 1) * P))
                            if qw_f32r:
                                nc.vector.tensor_copy(qtr[sl], src)
                            else:
                                _split_hilo(nc, qthi[sl], qtlo[sl], src,
                                            hi_on_scalar=True)

                def emit_finish(st_):
                    # deferred tail of a q-block. P^T is taken from the
                    # UNNORMALIZED exp chunks (no invz dependency); the 1/Z
                    # scale folds into the ctx PSUM->SBUF copy. The weights
                    # chunks are scaled in place afterwards for their DMA.
                    qb, wchunks, invz = st_
                    ptiles = []
                    for kc in range(NKC):
                        wo = wchunks[kc]
                        pt = PS_G.tile([P, 512], f32, tag="gen")
                        for j in range(4):
                            nc.tensor.transpose(
                                pt[:, j * P:(j + 1) * P],
                                wo[:, j * P:(j + 1) * P],
                                ident[:],
                            )
                        ptt = PTP.tile([P, 512], bf16, tag="pt")
                        nc.scalar.copy(ptt[:], pt[:])
                        ptiles.append(ptt)

                    for vc in range(NVC):
                        pc = PS_G.tile([P, 512], f32, tag="gen")
                        for kt in range(KT):
                            lhsT = ptiles[kt // 4][:, (kt % 4) * P:
                                                   (kt % 4 + 1) * P]
                            nc.tensor.matmul(
                                pc[:], lhsT,
                                vb[:, kt, vc * 512:(vc + 1) * 512],
                                start=(kt == 0), stop=(kt == KT - 1),
                            )
                        cx = CXP.tile([P, 512], f32, tag="cx")
                        nc.vector.tensor_scalar_mul(cx[:], pc[:], invz[:])
                        nc.sync.dma_start(
                            cx_out[qb * P:(qb + 1) * P,
                                   vc * 512:(vc + 1) * 512],
                            cx[:],
                        )

                    for kc in range(NKC):
                        wo = wchunks[kc]
                        nc.vector.tensor_scalar_mul(wo[:], wo[:], invz[:])
                        nc.sync.dma_start(
                            wt_out[qb * P:(qb + 1) * P,
                                   kc * 512:(kc + 1) * 512],
                            wo[:],
                        )

                pending = None
                emit_q_phase(0)
                emit_k_tiles(range(0, 4))
                emit_w_setup()
                for sb in range(NSB):
                    # -- qW^T[e, q] for this superblock --
                    for et in range(ET):
                        pq = PS_G.tile([P, SBQ], f32, tag="gen")
                        for dt in range(DT):
                            if qw_f32r:
                                mms = [(wr[:, dt, et * P:(et + 1) * P],
                                        qtr[:, dt, :])]
                            else:
                                lw_hi = whi[:, dt, et * P:(et + 1) * P]
                                lw_lo = wlo[:, dt, et * P:(et + 1) * P]
                                mms = [(lw_hi, qthi[:, dt, :]),
                                       (lw_hi, qtlo[:, dt, :]),
                                       (lw_lo, qthi[:, dt, :])]
                            for mi, (l, r) in enumerate(mms):
                                nc.tensor.matmul(
                                    pq[:], l, r,
                                    start=(dt == 0 and mi == 0),
                                    stop=(dt == DT - 1 and mi == len(mms) - 1),
                                )
                        if s_mode in ("bf16x3", "f32r2"):
                            _split_hilo(nc, qwhi[:, et, :], qwlo[:, et, :],
                                        pq[:], hi_on_scalar=True)
                        else:
                            nc.vector.tensor_copy(qwr[:, et, :], pq[:])

                    if sb == 0:
                        # rest of K + V land while qW(0) runs on PE
                        emit_k_tiles(range(4, KT))
                        emit_v_setup()

                    # -- per q-block: S, softmax, weights, P^T, ctx --
                    for qt in range(4):
                        qb = sb * 4 + qt
                        qq = qt * P

                        schunks = []
                        negmax = ST.tile([P, NKC], f32, tag="negmax")
                        for kc in range(NKC):
                            ps = PS_S.tile([P, 512], f32, tag="schunk")
                            ksl = (slice(None), None,
                                   slice(kc * 512, (kc + 1) * 512))
                            for et in range(ET):
                                if s_mode == "bf16x3":
                                    r_hi = kthi[:, et, kc * 512:(kc + 1) * 512]
                                    r_lo = ktlo[:, et, kc * 512:(kc + 1) * 512]
                                    mms = [(qwhi[:, et, qq:qq + P], r_hi),
                                           (qwhi[:, et, qq:qq + P], r_lo),
                                           (qwlo[:, et, qq:qq + P], r_hi)]
                                elif s_mode == "f32r2":
                                    r = ktr[:, et, kc * 512:(kc + 1) * 512]
                                    mms = [(qwhi[:, et, qq:qq + P], r),
                                           (qwlo[:, et, qq:qq + P], r)]
                                else:
                                    r = ktr[:, et, kc * 512:(kc + 1) * 512]
                                    mms = [(qwr[:, et, qq:qq + P], r)]
                                for mi, (l, r_) in enumerate(mms):
                                    nc.tensor.matmul(
                                        ps[:], l, r_,
                                        start=(et == 0 and mi == 0),
                                        stop=(et == ET - 1 and
                                              mi == len(mms) - 1),
                                    )
                            nc.vector.reduce_max(negmax[:, kc:kc + 1], ps[:],
                                                 axis=AX.X, negate=True)
                            schunks.append(ps)

                        # row stats:  -m = min(negmax);  Z = sum exp(s - m)
                        nmin = ST.tile([P, 1], f32, tag="nmin")
                        nc.vector.tensor_reduce(nmin[:], negmax[:], axis=AX.X,
                                                op=ALU.min)
                        zparts = ST.tile([P, NKC], f32, tag="zparts")
                        wchunks = []
                        for kc in range(NKC):
                            wo = WO.tile([P, 512], f32, tag="wout")
                            nc.scalar.activation(wo[:], schunks[kc][:], AF.Exp,
                                                 bias=nmin[:], scale=1.0,
                                                 accum_out=zparts[:, kc:kc + 1])
                            wchunks.append(wo)
                        z = ST.tile([P, 1], f32, tag="z")
                        nc.vector.reduce_sum(z[:], zparts[:], axis=AX.X)
                        invz = ST.tile([P, 1], f32, tag="invz")
                        nc.vector.reciprocal(invz[:], z[:])

                        # finish the PREVIOUS block while this one's softmax
                        # runs on DVE/ACT (keeps PE fed)
                        if pending is not None:
                            emit_finish(pending)
                        pending = (qb, wchunks, invz)

                        if qt == 0 and sb + 1 < NSB:
                            emit_q_phase(sb + 1)

                emit_finish(pending)

    nc.compile()
    return nc


_NC_CACHE = None


def _get_nc():
    global _NC_CACHE
    if _NC_CACHE is None:
        _NC_CACHE = build_kernel()
    return _NC_CACHE


def kernel(**inputs):
    q = np.ascontiguousarray(np.asarray(inputs["query"], dtype=np.float32))
    k = np.ascontiguousarray(np.asarray(inputs["key"], dtype=np.float32))
    v = np.ascontiguousarray(np.asarray(inputs["value"], dtype=np.float32))
    W = np.ascontiguousarray(np.asarray(inputs["W"], dtype=np.float32))
    bias = np.ascontiguousarray(np.asarray(inputs["bias"], dtype=np.float32))

    nc = _get_nc()
    in_maps = [
        {"query": q[i], "key": k[i], "value": v[i], "W": W, "bias": bias}
        for i in range(B)
    ]
    res = run_bass_kernel_spmd(nc, in_maps, core_ids=list(range(B)))
    weights = np.stack([res.results[i]["weights"] for i in range(B)])
    ctx = np.stack([res.results[i]["ctx"] for i in range(B)])
    return (weights, ctx)


if __name__ == "__main__":
    nc = build_kernel()
    print("kernel built ok")


# revision 43
# speedup vs baseline: 1.0171x; 1.0109x over previous
"""Trainium2 Bass kernel for bilinear-attention (weights softmax + context).

reference:
    qW = query @ W                      [B, Lq, D]
    scores = qW @ key^T + bias          [B, Lq, Lk]   (bias cancels in softmax)
    weights = softmax(scores, -1)       [B, Lq, Lk]
    ctx = weights @ value               [B, Lq, Dv]
    returns (weights, ctx)

Sharding: data-parallel over batch B=8 -> one batch element per NeuronCore.

Numerics per core:
  - qW^T: bf16 hi/lo 3-pass matmul (x ~ hi + lo; x@y ~ hh + hl + lh), fp32 PSUM.
  - scores: selectable S_MODE:
      "bf16x3": bf16 hi/lo 3-pass both sides (most accurate)
      "f32r2" : stationary qW^T split into two float32r parts, K^T single f32r
      "f32r1" : single-pass float32r (fastest)
  - softmax: chunk max (negated) -> exp(s - m) with accumulated Z -> 1/Z scale
  - ctx: single-pass bf16 (weights^T via PE transpose x V)
"""
import sys
import os

for _p in ("/opt/trn_rl_repo", "/root/.axon_site/_ro/trn_rl_repo"):
    if os.path.isdir(_p) and _p not in sys.path:
        sys.path.insert(0, _p)

import numpy as np
import concourse.bass as bass
import concourse.mybir as mybir
import concourse.tile as tile
from concourse import bacc
from concourse.bass_utils import run_bass_kernel_spmd
from concourse.masks import make_identity

f32 = mybir.dt.float32
f32r = mybir.dt.float32r
bf16 = mybir.dt.bfloat16
AF = mybir.ActivationFunctionType
AX = mybir.AxisListType
ALU = mybir.AluOpType

# Problem shape (hardcoded; one batch element per core)
B, LQ, LK, D, DV = 8, 2048, 2048, 1024, 1024
P = 128                      # partitions
DT = D // P                  # 8 d-tiles
ET = D // P                  # 8 e-tiles
KT = LK // P                 # 16 k-tiles
NQB = LQ // P                # 16 q-blocks
NSB = 4                      # q-superblocks (512 q each) for qW phase
SBQ = LQ // NSB              # 512
NKC = LK // 512              # 4 k-chunks of 512
NVC = DV // 512              # 2 v-chunks of 512

S_MODE = os.environ.get("S_MODE", "f32rall")


def _split_hilo(nc, hi_slice, lo_slice, src_f32, hi_on_scalar=False):
    """hi = round(src); lo = round(src - hi). src may be PSUM or SBUF f32."""
    if hi_on_scalar:
        nc.scalar.copy(hi_slice, src_f32)
    else:
        nc.vector.tensor_copy(hi_slice, src_f32)
    nc.vector.tensor_sub(lo_slice, src_f32, hi_slice)


def build_kernel(s_mode=None):
    s_mode = s_mode or S_MODE
    assert s_mode in ("bf16x3", "f32r2", "f32r1", "f32rall")
    qw_f32r = s_mode == "f32rall"       # qW single-pass f32r too

    nc = bacc.Bacc("TRN2", target_bir_lowering=False, debug=False)

    q_in = nc.declare_dram_parameter("query", [LQ, D], f32, isOutput=False)
    k_in = nc.declare_dram_parameter("key", [LK, D], f32, isOutput=False)
    v_in = nc.declare_dram_parameter("value", [LK, DV], f32, isOutput=False)
    w_in = nc.declare_dram_parameter("W", [D, D], f32, isOutput=False)
    nc.declare_dram_parameter("bias", [1], f32, isOutput=False)  # softmax-invariant
    wt_out = nc.declare_dram_parameter("weights", [LQ, LK], f32, isOutput=True)
    cx_out = nc.declare_dram_parameter("ctx", [LQ, DV], f32, isOutput=True)

    with tile.TileContext(nc) as tc:
        with tc.tile_pool(name="resident", bufs=1) as RP, \
             tc.tile_pool(name="qf", bufs=2) as QF, \
             tc.tile_pool(name="qt", bufs=1) as QT, \
             tc.tile_pool(name="ps_s", bufs=4, space="PSUM") as PS_S, \
             tc.tile_pool(name="ps_gen", bufs=4, space="PSUM") as PS_G:

            ident = RP.tile([P, P], f32)
            make_identity(nc, ident[:])
            identb = RP.tile([P, P], bf16)
            nc.vector.tensor_copy(identb[:], ident[:])

            # resident operand arrays
            if qw_f32r:
                wr = RP.tile([P, DT, D], f32r)   # W[d, e] (d on partitions)
            else:
                whi = RP.tile([P, DT, D], bf16)
                wlo = RP.tile([P, DT, D], bf16)
            if s_mode == "bf16x3":
                kthi = RP.tile([P, ET, LK], bf16)  # K^T[e, k] (e on partitions)
                ktlo = RP.tile([P, ET, LK], bf16)
            else:
                ktr = RP.tile([P, ET, LK], f32r)
            vb = RP.tile([P, KT, DV], bf16)      # V[k, v]   (k on partitions)

            # Q^T for one superblock (written per-superblock in main loop)
            if qw_f32r:
                qtr = QT.tile([P, DT, SBQ], f32r, tag="qtr")
            else:
                qthi = QT.tile([P, DT, SBQ], bf16, tag="qthi")
                qtlo = QT.tile([P, DT, SBQ], bf16, tag="qtlo")

            # ---------------- main ----------------
            with tc.tile_pool(name="qwt", bufs=1) as QWT, \
                 tc.tile_pool(name="wout", bufs=8) as WO, \
                 tc.tile_pool(name="ptp", bufs=5) as PTP, \
                 tc.tile_pool(name="cxp", bufs=2) as CXP, \
                 tc.tile_pool(name="stats", bufs=2) as ST, \
                 tc.tile_pool(name="setup", bufs=3) as SP:

                def emit_w_setup():
                    # W gates the qW matmuls of superblock 0 - do it first
                    for c in range(DT):
                        wf = SP.tile([P, 1, D], f32, tag="wvf")
                        nc.sync.dma_start(
                            wf[:],
                            w_in.rearrange("(t p) e -> p t e",
                                           p=P)[:, c:c + 1, :],
                        )
                        if qw_f32r:
                            nc.vector.tensor_copy(wr[:, c:c + 1, :], wf[:])
                        else:
                            _split_hilo(nc, whi[:, c:c + 1, :],
                                        wlo[:, c:c + 1, :], wf[:],
                                        hi_on_scalar=True)

                def emit_k_tiles(kts):
                    # K^T via PE transposes; copies on ACT
                    for kt in kts:
                        kf = QF.tile([P, D], f32, tag="qf")
                        nc.sync.dma_start(kf[:], k_in[kt * P:(kt + 1) * P, :])
                        for g in range(2):  # groups of 4 e-tiles
                            pt = PS_G.tile([P, 512], f32, tag="gen")
                            for j in range(4):
                                et = g * 4 + j
                                nc.tensor.transpose(
                                    pt[:, j * P:(j + 1) * P],
                                    kf[:, et * P:(et + 1) * P],
                                    ident[:],
                                )
                            src = pt[:].rearrange("p (a b) -> p a b", a=4)
                            sl = (slice(None), slice(g * 4, (g + 1) * 4),
                                  slice(kt * P, (kt + 1) * P))
                            if s_mode == "bf16x3":
                                _split_hilo(nc, kthi[sl], ktlo[sl], src,
                                            hi_on_scalar=True)
                            else:
                                nc.scalar.copy(ktr[sl], src)

                def emit_v_setup():
                    # V rounded to bf16 on DVE (needed only at first ctx)
                    for c in range(KT):
                        vf = SP.tile([P, 1, DV], f32, tag="wvf")
                        nc.sync.dma_start(
                            vf[:],
                            v_in.rearrange("(t p) v -> p t v",
                                           p=P)[:, c:c + 1, :],
                        )
                        nc.vector.tensor_copy(vb[:, c:c + 1, :], vf[:])

                if s_mode == "bf16x3":
                    qwhi = QWT.tile([P, ET, SBQ], bf16, tag="qwhi")
                    qwlo = QWT.tile([P, ET, SBQ], bf16, tag="qwlo")
                elif s_mode == "f32r2":
                    qwhi = QWT.tile([P, ET, SBQ], f32r, tag="qwhi")
                    qwlo = QWT.tile([P, ET, SBQ], f32r, tag="qwlo")
                else:
                    qwr = QWT.tile([P, ET, SBQ], f32r, tag="qwr")

                def emit_q_phase(sb):
                    # Q^T (hi/lo) for superblock sb
                    for qt in range(4):
                        qb = sb * 4 + qt
                        qf = QF.tile([P, D], f32, tag="qf")
                        nc.sync.dma_start(qf[:], q_in[qb * P:(qb + 1) * P, :])
                        for g in range(2):
                            pt = PS_G.tile([P, 512], f32, tag="gen")
                            for j in range(4):
                                dt = g * 4 + j
                                nc.tensor.transpose(
                                    pt[:, j * P:(j + 1) * P],
                                    qf[:, dt * P:(dt + 1) * P],
                                    ident[:],
                                )
                            src = pt[:].rearrange("p (a b) -> p a b", a=4)
                            sl = (slice(None), slice(g * 4, (g + 1) * 4),
                                  slice(qt * P, (qt + 1) * P))
                            if qw_f32r:
                                nc.vector.tensor_copy(qtr[sl], src)
                            else:
                                _split_hilo(nc, qthi[sl], qtlo[sl], src,
                                            hi_on_scalar=True)

                def emit_finish(st_):
                    # deferred tail of a q-block. P^T is taken from the
                    # UNNORMALIZED exp chunks (no invz dependency); the 1/Z
                    # scale folds into the ctx PSUM->SBUF copy. The weights
                    # chunks are scaled in place afterwards for their DMA.
                    qb, wchunks, invz = st_
                    ptiles = []
                    for kc in range(NKC):
                        wo = wchunks[kc]
                        pt = PS_G.tile([P, 512], f32, tag="gen")
                        for j in range(4):
                            nc.tensor.transpose(
                                pt[:, j * P:(j + 1) * P],
                                wo[:, j * P:(j + 1) * P],
                                ident[:],
                            )
                        ptt = PTP.tile([P, 512], bf16, tag="pt")
                        nc.scalar.copy(ptt[:], pt[:])
                        ptiles.append(ptt)

                    for vc in range(NVC):
                        pc = PS_G.tile([P, 512], f32, tag="gen")
                        for kt in range(KT):
                            lhsT = ptiles[kt // 4][:, (kt % 4) * P:
                                                   (kt % 4 + 1) * P]
                            nc.tensor.matmul(
                                pc[:], lhsT,
                                vb[:, kt, vc * 512:(vc + 1) * 512],
                                start=(kt == 0), stop=(kt == KT - 1),
                            )
                        cx = CXP.tile([P, 512], f32, tag="cx")
                        nc.vector.tensor_scalar_mul(cx[:], pc[:], invz[:])
                        nc.sync.dma_start(
                            cx_out[qb * P:(qb + 1) * P,
                                   vc * 512:(vc + 1) * 512],
                            cx[:],
                        )

                    for kc in range(NKC):
                        wo = wchunks[kc]
                        nc.vector.tensor_scalar_mul(wo[:], wo[:], invz[:])
                        nc.sync.dma_start(
                            wt_out[qb * P:(qb + 1) * P,
                                   kc * 512:(kc + 1) * 512],
                            wo[:],
                        )

                pending = None
                emit_q_phase(0)
                emit_k_tiles(range(0, 4))
                emit_w_setup()
                for sb in range(NSB):
                    # -- qW^T[e, q] for this superblock --
                    for et in range(ET):
                        pq = PS_G.tile([P, SBQ], f32, tag="gen")
                        for dt in range(DT):
                            if qw_f32r:
                                mms = [(wr[:, dt, et * P:(et + 1) * P],
                                        qtr[:, dt, :])]
                            else:
                                lw_hi = whi[:, dt, et * P:(et + 1) * P]
                                lw_lo = wlo[:, dt, et * P:(et + 1) * P]
                                mms = [(lw_hi, qthi[:, dt, :]),
                                       (lw_hi, qtlo[:, dt, :]),
                                       (lw_lo, qthi[:, dt, :])]
                            for mi, (l, r) in enumerate(mms):
                                nc.tensor.matmul(
                                    pq[:], l, r,
                                    start=(dt == 0 and mi == 0),
                                    stop=(dt == DT - 1 and mi == len(mms) - 1),
                                )
                        if s_mode in ("bf16x3", "f32r2"):
                            _split_hilo(nc, qwhi[:, et, :], qwlo[:, et, :],
                                        pq[:], hi_on_scalar=True)
                        else:
                            nc.vector.tensor_copy(qwr[:, et, :], pq[:])

                    if sb == 0:
                        # rest of K + V land while qW(0) runs on PE
                        emit_k_tiles(range(4, KT))
                        emit_v_setup()

                    # -- per q-block: S, softmax, weights, P^T, ctx --
                    for qt in range(4):
                        qb = sb * 4 + qt
                        qq = qt * P

                        schunks = []
                        negmax = ST.tile([P, NKC], f32, tag="negmax")
                        for kc in range(NKC):
                            ps = PS_S.tile([P, 512], f32, tag="schunk")
                            ksl = (slice(None), None,
                                   slice(kc * 512, (kc + 1) * 512))
                            for et in range(ET):
                                if s_mode == "bf16x3":
                                    r_hi = kthi[:, et, kc * 512:(kc + 1) * 512]
                                    r_lo = ktlo[:, et, kc * 512:(kc + 1) * 512]
                                    mms = [(qwhi[:, et, qq:qq + P], r_hi),
                                           (qwhi[:, et, qq:qq + P], r_lo),
                                           (qwlo[:, et, qq:qq + P], r_hi)]
                                elif s_mode == "f32r2":
                                    r = ktr[:, et, kc * 512:(kc + 1) * 512]
                                    mms = [(qwhi[:, et, qq:qq + P], r),
                                           (qwlo[:, et, qq:qq + P], r)]
                                else:
                                    r = ktr[:, et, kc * 512:(kc + 1) * 512]
                                    mms = [(qwr[:, et, qq:qq + P], r)]
                                for mi, (l, r_) in enumerate(mms):
                                    nc.tensor.matmul(
                                        ps[:], l, r_,
                                        start=(et == 0 and mi == 0),
                                        stop=(et == ET - 1 and
                                              mi == len(mms) - 1),
                                    )
                            nc.vector.reduce_max(negmax[:, kc:kc + 1], ps[:],
                                                 axis=AX.X, negate=True)
                            schunks.append(ps)

                        # row stats:  -m = min(negmax);  Z = sum exp(s - m)
                        nmin = ST.tile([P, 1], f32, tag="nmin")
                        nc.vector.tensor_reduce(nmin[:], negmax[:], axis=AX.X,
                                                op=ALU.min)
                        zparts = ST.tile([P, NKC], f32, tag="zparts")
                        wchunks = []
                        for kc in range(NKC):
                            wo = WO.tile([P, 512], f32, tag="wout")
                            nc.scalar.activation(wo[:], schunks[kc][:], AF.Exp,
                                                 bias=nmin[:], scale=1.0,
                                                 accum_out=zparts[:, kc:kc + 1])
                            wchunks.append(wo)
                        z = ST.tile([P, 1], f32, tag="z")
                        nc.vector.reduce_sum(z[:], zparts[:], axis=AX.X)
                        invz = ST.tile([P, 1], f32, tag="invz")
                        nc.vector.reciprocal(invz[:], z[:])

                        # finish the PREVIOUS block while this one's softmax
                        # runs on DVE/ACT (keeps PE fed)
                        if pending is not None:
                            emit_finish(pending)
                        pending = (qb, wchunks, invz)

                        if qt == 0 and sb + 1 < NSB:
                            emit_q_phase(sb + 1)

                emit_finish(pending)

    nc.compile()
    return nc


_NC_CACHE = None


def _get_nc():
    global _NC_CACHE
    if _NC_CACHE is None:
        _NC_CACHE = build_kernel()
    return _NC_CACHE


def kernel(**inputs):
    q = np.ascontiguousarray(np.asarray(inputs["query"], dtype=np.float32))
    k = np.ascontiguousarray(np.asarray(inputs["key"], dtype=np.float32))
    v = np.ascontiguousarray(np.asarray(inputs["value"], dtype=np.float32))
    W = np.ascontiguousarray(np.asarray(inputs["W"], dtype=np.float32))
    bias = np.ascontiguousarray(np.asarray(inputs["bias"], dtype=np.float32))

    nc = _get_nc()
    in_maps = [
        {"query": q[i], "key": k[i], "value": v[i], "W": W, "bias": bias}
        for i in range(B)
    ]
    res = run_bass_kernel_spmd(nc, in_maps, core_ids=list(range(B)))
    weights = np.stack([res.results[i]["weights"] for i in range(B)])
    ctx = np.stack([res.results[i]["ctx"] for i in range(B)])
    return (weights, ctx)


if __name__ == "__main__":
    nc = build_kernel()
    print("kernel built ok")


# revision 46
# speedup vs baseline: 1.0257x; 1.0085x over previous
"""Trainium2 Bass kernel for bilinear-attention (weights softmax + context).

reference:
    qW = query @ W                      [B, Lq, D]
    scores = qW @ key^T + bias          [B, Lq, Lk]   (bias cancels in softmax)
    weights = softmax(scores, -1)       [B, Lq, Lk]
    ctx = weights @ value               [B, Lq, Dv]
    returns (weights, ctx)

Sharding: data-parallel over batch B=8 -> one batch element per NeuronCore.

Numerics per core:
  - qW^T: bf16 hi/lo 3-pass matmul (x ~ hi + lo; x@y ~ hh + hl + lh), fp32 PSUM.
  - scores: selectable S_MODE:
      "bf16x3": bf16 hi/lo 3-pass both sides (most accurate)
      "f32r2" : stationary qW^T split into two float32r parts, K^T single f32r
      "f32r1" : single-pass float32r (fastest)
  - softmax: chunk max (negated) -> exp(s - m) with accumulated Z -> 1/Z scale
  - ctx: single-pass bf16 (weights^T via PE transpose x V)
"""
import sys
import os

for _p in ("/opt/trn_rl_repo", "/root/.axon_site/_ro/trn_rl_repo"):
    if os.path.isdir(_p) and _p not in sys.path:
        sys.path.insert(0, _p)

import numpy as np
import concourse.bass as bass
import concourse.mybir as mybir
import concourse.tile as tile
from concourse import bacc
from concourse.bass_utils import run_bass_kernel_spmd
from concourse.masks import make_identity

f32 = mybir.dt.float32
f32r = mybir.dt.float32r
bf16 = mybir.dt.bfloat16
AF = mybir.ActivationFunctionType
AX = mybir.AxisListType
ALU = mybir.AluOpType

# Problem shape (hardcoded; one batch element per core)
B, LQ, LK, D, DV = 8, 2048, 2048, 1024, 1024
P = 128                      # partitions
DT = D // P                  # 8 d-tiles
ET = D // P                  # 8 e-tiles
KT = LK // P                 # 16 k-tiles
NQB = LQ // P                # 16 q-blocks
NSB = 4                      # q-superblocks (512 q each) for qW phase
SBQ = LQ // NSB              # 512
NKC = LK // 512              # 4 k-chunks of 512
NVC = DV // 512              # 2 v-chunks of 512

S_MODE = os.environ.get("S_MODE", "f32rall")


def _split_hilo(nc, hi_slice, lo_slice, src_f32, hi_on_scalar=False):
    """hi = round(src); lo = round(src - hi). src may be PSUM or SBUF f32."""
    if hi_on_scalar:
        nc.scalar.copy(hi_slice, src_f32)
    else:
        nc.vector.tensor_copy(hi_slice, src_f32)
    nc.vector.tensor_sub(lo_slice, src_f32, hi_slice)


def build_kernel(s_mode=None):
    s_mode = s_mode or S_MODE
    assert s_mode in ("bf16x3", "f32r2", "f32r1", "f32rall")
    qw_f32r = s_mode == "f32rall"       # qW single-pass f32r too

    nc = bacc.Bacc("TRN2", target_bir_lowering=False, debug=False)

    q_in = nc.declare_dram_parameter("query", [LQ, D], f32, isOutput=False)
    k_in = nc.declare_dram_parameter("key", [LK, D], f32, isOutput=False)
    v_in = nc.declare_dram_parameter("value", [LK, DV], f32, isOutput=False)
    w_in = nc.declare_dram_parameter("W", [D, D], f32, isOutput=False)
    nc.declare_dram_parameter("bias", [1], f32, isOutput=False)  # softmax-invariant
    wt_out = nc.declare_dram_parameter("weights", [LQ, LK], f32, isOutput=True)
    cx_out = nc.declare_dram_parameter("ctx", [LQ, DV], f32, isOutput=True)

    with tile.TileContext(nc) as tc:
        with tc.tile_pool(name="resident", bufs=1) as RP, \
             tc.tile_pool(name="qf", bufs=2) as QF, \
             tc.tile_pool(name="qt", bufs=1) as QT, \
             tc.tile_pool(name="ps_s", bufs=4, space="PSUM") as PS_S, \
             tc.tile_pool(name="ps_gen", bufs=4, space="PSUM") as PS_G:

            ident = RP.tile([P, P], f32)
            make_identity(nc, ident[:])
            identb = RP.tile([P, P], bf16)
            nc.vector.tensor_copy(identb[:], ident[:])

            # resident operand arrays
            if qw_f32r:
                wr = RP.tile([P, DT, D], f32r)   # W[d, e] (d on partitions)
            else:
                whi = RP.tile([P, DT, D], bf16)
                wlo = RP.tile([P, DT, D], bf16)
            if s_mode == "bf16x3":
                kthi = RP.tile([P, ET, LK], bf16)  # K^T[e, k] (e on partitions)
                ktlo = RP.tile([P, ET, LK], bf16)
            else:
                ktr = RP.tile([P, ET, LK], f32r)
            vb = RP.tile([P, KT, DV], bf16)      # V[k, v]   (k on partitions)

            # Q^T for one superblock (written per-superblock in main loop)
            if qw_f32r:
                qtr = QT.tile([P, DT, SBQ], f32r, tag="qtr")
            else:
                qthi = QT.tile([P, DT, SBQ], bf16, tag="qthi")
                qtlo = QT.tile([P, DT, SBQ], bf16, tag="qtlo")

            # ---------------- main ----------------
            with tc.tile_pool(name="qwt", bufs=1) as QWT, \
                 tc.tile_pool(name="wout", bufs=8) as WO, \
                 tc.tile_pool(name="ptp", bufs=5) as PTP, \
                 tc.tile_pool(name="cxp", bufs=2) as CXP, \
                 tc.tile_pool(name="stats", bufs=2) as ST, \
                 tc.tile_pool(name="setup", bufs=3) as SP:

                def emit_w_setup():
                    # W gates the qW matmuls of superblock 0 - do it first
                    for c in range(DT):
                        wf = SP.tile([P, 1, D], f32, tag="wvf")
                        nc.sync.dma_start(
                            wf[:],
                            w_in.rearrange("(t p) e -> p t e",
                                           p=P)[:, c:c + 1, :],
                        )
                        if qw_f32r:
                            nc.vector.tensor_copy(wr[:, c:c + 1, :], wf[:])
                        else:
                            _split_hilo(nc, whi[:, c:c + 1, :],
                                        wlo[:, c:c + 1, :], wf[:],
                                        hi_on_scalar=True)

                def emit_k_tiles(kts):
                    # K^T via PE transposes; copies on ACT
                    for kt in kts:
                        kf = QF.tile([P, D], f32, tag="qf")
                        nc.sync.dma_start(kf[:], k_in[kt * P:(kt + 1) * P, :])
                        for g in range(2):  # groups of 4 e-tiles
                            pt = PS_G.tile([P, 512], f32, tag="gen")
                            for j in range(4):
                                et = g * 4 + j
                                nc.tensor.transpose(
                                    pt[:, j * P:(j + 1) * P],
                                    kf[:, et * P:(et + 1) * P],
                                    ident[:],
                                )
                            src = pt[:].rearrange("p (a b) -> p a b", a=4)
                            sl = (slice(None), slice(g * 4, (g + 1) * 4),
                                  slice(kt * P, (kt + 1) * P))
                            if s_mode == "bf16x3":
                                _split_hilo(nc, kthi[sl], ktlo[sl], src,
                                            hi_on_scalar=True)
                            else:
                                nc.scalar.copy(ktr[sl], src)

                def emit_v_setup():
                    # V rounded to bf16 on DVE (needed only at first ctx)
                    for c in range(KT):
                        vf = SP.tile([P, 1, DV], f32, tag="wvf")
                        nc.sync.dma_start(
                            vf[:],
                            v_in.rearrange("(t p) v -> p t v",
                                           p=P)[:, c:c + 1, :],
                        )
                        nc.vector.tensor_copy(vb[:, c:c + 1, :], vf[:])

                if s_mode == "bf16x3":
                    qwhi = QWT.tile([P, ET, SBQ], bf16, tag="qwhi")
                    qwlo = QWT.tile([P, ET, SBQ], bf16, tag="qwlo")
                elif s_mode == "f32r2":
                    qwhi = QWT.tile([P, ET, SBQ], f32r, tag="qwhi")
                    qwlo = QWT.tile([P, ET, SBQ], f32r, tag="qwlo")
                else:
                    qwr = QWT.tile([P, ET, SBQ], f32r, tag="qwr")

                def emit_q_phase(sb):
                    # Q^T (hi/lo) for superblock sb
                    for qt in range(4):
                        qb = sb * 4 + qt
                        qf = QF.tile([P, D], f32, tag="qf")
                        nc.sync.dma_start(qf[:], q_in[qb * P:(qb + 1) * P, :])
                        for g in range(2):
                            pt = PS_G.tile([P, 512], f32, tag="gen")
                            for j in range(4):
                                dt = g * 4 + j
                                nc.tensor.transpose(
                                    pt[:, j * P:(j + 1) * P],
                                    qf[:, dt * P:(dt + 1) * P],
                                    ident[:],
                                )
                            src = pt[:].rearrange("p (a b) -> p a b", a=4)
                            sl = (slice(None), slice(g * 4, (g + 1) * 4),
                                  slice(qt * P, (qt + 1) * P))
                            if qw_f32r:
                                nc.vector.tensor_copy(qtr[sl], src)
                            else:
                                _split_hilo(nc, qthi[sl], qtlo[sl], src,
                                            hi_on_scalar=True)

                def emit_finish(st_):
                    # deferred tail of a q-block. P^T is taken from the
                    # UNNORMALIZED exp chunks (no invz dependency); the 1/Z
                    # scale folds into the ctx PSUM->SBUF copy. The weights
                    # chunks are scaled in place afterwards for their DMA.
                    qb, wchunks, invz = st_
                    ptiles = []
                    for kc in range(NKC):
                        wo = wchunks[kc]
                        pt = PS_G.tile([P, 512], f32, tag="gen")
                        for j in range(4):
                            nc.tensor.transpose(
                                pt[:, j * P:(j + 1) * P],
                                wo[:, j * P:(j + 1) * P],
                                ident[:],
                            )
                        ptt = PTP.tile([P, 512], bf16, tag="pt")
                        nc.scalar.copy(ptt[:], pt[:])
                        ptiles.append(ptt)

                    for vc in range(NVC):
                        pc = PS_G.tile([P, 512], f32, tag="gen")
                        for kt in range(KT):
                            lhsT = ptiles[kt // 4][:, (kt % 4) * P:
                                                   (kt % 4 + 1) * P]
                            nc.tensor.matmul(
                                pc[:], lhsT,
                                vb[:, kt, vc * 512:(vc + 1) * 512],
                                start=(kt == 0), stop=(kt == KT - 1),
                            )
                        cx = CXP.tile([P, 512], f32, tag="cx")
                        nc.vector.tensor_scalar_mul(cx[:], pc[:], invz[:])
                        nc.sync.dma_start(
                            cx_out[qb * P:(qb + 1) * P,
                                   vc * 512:(vc + 1) * 512],
                            cx[:],
                        )

                    for kc in range(NKC):
                        wo = wchunks[kc]
                        nc.vector.tensor_scalar_mul(wo[:], wo[:], invz[:])
                        nc.sync.dma_start(
                            wt_out[qb * P:(qb + 1) * P,
                                   kc * 512:(kc + 1) * 512],
                            wo[:],
                        )

                pending = None
                emit_q_phase(0)
                emit_k_tiles(range(0, 4))
                emit_w_setup()
                for sb in range(NSB):
                    # -- qW^T[e, q] for this superblock --
                    for et in range(ET):
                        pq = PS_G.tile([P, SBQ], f32, tag="gen")
                        for dt in range(DT):
                            if qw_f32r:
                                mms = [(wr[:, dt, et * P:(et + 1) * P],
                                        qtr[:, dt, :])]
                            else:
                                lw_hi = whi[:, dt, et * P:(et + 1) * P]
                                lw_lo = wlo[:, dt, et * P:(et + 1) * P]
                                mms = [(lw_hi, qthi[:, dt, :]),
                                       (lw_hi, qtlo[:, dt, :]),
                                       (lw_lo, qthi[:, dt, :])]
                            for mi, (l, r) in enumerate(mms):
                                nc.tensor.matmul(
                                    pq[:], l, r,
                                    start=(dt == 0 and mi == 0),
                                    stop=(dt == DT - 1 and mi == len(mms) - 1),
                                )
                        if s_mode in ("bf16x3", "f32r2"):
                            _split_hilo(nc, qwhi[:, et, :], qwlo[:, et, :],
                                        pq[:], hi_on_scalar=True)
                        else:
                            nc.vector.tensor_copy(qwr[:, et, :], pq[:])

                    if sb == 0:
                        # rest of K + V land while qW(0) runs on PE.
                        # Stage K tiles through the (setup-idle) wout pool as
                        # [128,512] halves for deeper DMA pipelining.
                        for kt in range(4, KT):
                            kfa = WO.tile([P, 512], f32, tag="wout")
                            kfb = WO.tile([P, 512], f32, tag="wout")
                            nc.sync.dma_start(kfa[:], k_in[kt * P:(kt + 1) * P, 0:512])
                            nc.sync.dma_start(kfb[:], k_in[kt * P:(kt + 1) * P, 512:1024])
                            for g in range(2):
                                pt = PS_G.tile([P, 512], f32, tag="gen")
                                for j in range(4):
                                    et = g * 4 + j
                                    srcbuf = kfa if et < 4 else kfb
                                    nc.tensor.transpose(
                                        pt[:, j * P:(j + 1) * P],
                                        srcbuf[:, (et % 4) * P:(et % 4 + 1) * P],
                                        ident[:],
                                    )
                                src = pt[:].rearrange("p (a b) -> p a b", a=4)
                                sl = (slice(None), slice(g * 4, (g + 1) * 4),
                                      slice(kt * P, (kt + 1) * P))
                                if s_mode == "bf16x3":
                                    _split_hilo(nc, kthi[sl], ktlo[sl], src,
                                                hi_on_scalar=True)
                                else:
                                    nc.scalar.copy(ktr[sl], src)
                        emit_v_setup()

                    # -- per q-block: S, softmax, weights, P^T, ctx --
                    for qt in range(4):
                        qb = sb * 4 + qt
                        qq = qt * P

                        schunks = []
                        negmax = ST.tile([P, NKC], f32, tag="negmax")
                        for kc in range(NKC):
                            ps = PS_S.tile([P, 512], f32, tag="schunk")
                            ksl = (slice(None), None,
                                   slice(kc * 512, (kc + 1) * 512))
                            for et in range(ET):
                                if s_mode == "bf16x3":
                                    r_hi = kthi[:, et, kc * 512:(kc + 1) * 512]
                                    r_lo = ktlo[:, et, kc * 512:(kc + 1) * 512]
                                    mms = [(qwhi[:, et, qq:qq + P], r_hi),
                                           (qwhi[:, et, qq:qq + P], r_lo),
                                           (qwlo[:, et, qq:qq + P], r_hi)]
                                elif s_mode == "f32r2":
                                    r = ktr[:, et, kc * 512:(kc + 1) * 512]
                                    mms = [(qwhi[:, et, qq:qq + P], r),
                                           (qwlo[:, et, qq:qq + P], r)]
                                else:
                                    r = ktr[:, et, kc * 512:(kc + 1) * 512]
                                    mms = [(qwr[:, et, qq:qq + P], r)]
                                for mi, (l, r_) in enumerate(mms):
                                    nc.tensor.matmul(
                                        ps[:], l, r_,
                                        start=(et == 0 and mi == 0),
                                        stop=(et == ET - 1 and
                                              mi == len(mms) - 1),
                                    )
                            nc.vector.reduce_max(negmax[:, kc:kc + 1], ps[:],
                                                 axis=AX.X, negate=True)
                            schunks.append(ps)

                        # row stats:  -m = min(negmax);  Z = sum exp(s - m)
                        nmin = ST.tile([P, 1], f32, tag="nmin")
                        nc.vector.tensor_reduce(nmin[:], negmax[:], axis=AX.X,
                                                op=ALU.min)
                        zparts = ST.tile([P, NKC], f32, tag="zparts")
                        wchunks = []
                        for kc in range(NKC):
                            wo = WO.tile([P, 512], f32, tag="wout")
                            nc.scalar.activation(wo[:], schunks[kc][:], AF.Exp,
                                                 bias=nmin[:], scale=1.0,
                                                 accum_out=zparts[:, kc:kc + 1])
                            wchunks.append(wo)
                        z = ST.tile([P, 1], f32, tag="z")
                        nc.vector.reduce_sum(z[:], zparts[:], axis=AX.X)
                        invz = ST.tile([P, 1], f32, tag="invz")
                        nc.vector.reciprocal(invz[:], z[:])

                        # finish the PREVIOUS block while this one's softmax
                        # runs on DVE/ACT (keeps PE fed)
                        if pending is not None:
                            emit_finish(pending)
                        pending = (qb, wchunks, invz)

                        if qt == 0 and sb + 1 < NSB:
                            emit_q_phase(sb + 1)

                emit_finish(pending)

    nc.compile()
    return nc


_NC_CACHE = None


def _get_nc():
    global _NC_CACHE
    if _NC_CACHE is None:
        _NC_CACHE = build_kernel()
    return _NC_CACHE


def kernel(**inputs):
    q = np.ascontiguousarray(np.asarray(inputs["query"], dtype=np.float32))
    k = np.ascontiguousarray(np.asarray(inputs["key"], dtype=np.float32))
    v = np.ascontiguousarray(np.asarray(inputs["value"], dtype=np.float32))
    W = np.ascontiguousarray(np.asarray(inputs["W"], dtype=np.float32))
    bias = np.ascontiguousarray(np.asarray(inputs["bias"], dtype=np.float32))

    nc = _get_nc()
    in_maps = [
        {"query": q[i], "key": k[i], "value": v[i], "W": W, "bias": bias}
        for i in range(B)
    ]
    res = run_bass_kernel_spmd(nc, in_maps, core_ids=list(range(B)))
    weights = np.stack([res.results[i]["weights"] for i in range(B)])
    ctx = np.stack([res.results[i]["ctx"] for i in range(B)])
    return (weights, ctx)


if __name__ == "__main__":
    nc = build_kernel()
    print("kernel built ok")
